# revision 20
# baseline (speedup 1.0000x reference)
"""BiGraphConv (gather + SpMM segment-sum + linear) on 8 Trainium2 NeuronCores.

Strategy (1D output-row partition, staged replication):
  - Output rows are assigned to (core, block) bins on the host with a
    stratified deal (118 rows per 128-row block), equalizing per-(block,
    piece) edge counts across cores so the SPMD-uniform chunk counts
    carry ~8% padding instead of ~25%.  Each core owns its bins' edges;
    no inter-core reduction is needed.  The host unshuffles the output.
  - b_input is cast to bf16 on the host and uploaded sharded (1/8 per
    core).  Replication is staged: the table is split into 4 "pieces"
    (piece k = concat of every shard's k-th 3125-row sub-slice, 25000
    rows each, small enough for dma_gather's int16 indices), each
    materialized by its own AllGather.  Compute is piece-major, so
    piece-0 compute overlaps the remaining collectives.
  - Edge-row gathers use the GPSIMD dma_gather custom instruction (mlp
    ucode library): one op fetches ~3k 256B rows, amortizing the ~1us
    SWDGE descriptor-generation fixed cost.  Only gathers run on the
    Pool/SWDGE path; every regular DMA uses HWDGE.
  - Per piece, each block accumulates its chunks in its own PSUM bank
    (matmul start/stop state is per-bank: concurrent chains must not
    share one), then flushes into a per-block SBUF bf16 accumulator
    (ScalarE copy on piece 0, VectorE add after); the last flush writes
    bf16 and feeds the weight matmul directly: out[r,f] = Y2.T @ W.
  - Segment-sum via TensorE in bf16: per chunk of 128 edges,
    Y2[c,r] += sum_e G[e,c] * S_T[e,r] with S_T[e,r] = val_e*(row_e == r)
    built on VectorE with one fused tensor_scalar (is_equal then mult).
  - The bias rides the output PSUM accumulation as a rank-1 matmul
    (ones x bias); the output needs only a ScalarE copy (as bf16) before
    the store, and the host upcasts to f32.

kernel(**inputs) takes the FULL inputs and returns the FULL [100000,128]
output.  Self-contained: shapes/sharding are hardcoded.
"""

import numpy as np
import ml_dtypes

import concourse.bass as bass
import concourse.mybir as mybir
import concourse.tile as tile
from concourse import library_config
from concourse.bass_utils import run_bass_kernel_spmd
from concourse.library_overlay import lower_extended_insts

NA = 100000
NB = 100000
NE = 1600000
F = 128          # feature dim (both sides)
P = 128          # partitions / block rows / chunk size
N_CORES = 8
SHARD = NB // N_CORES                  # 12500 table rows per core
NP = 4                                 # table pieces (int16 idx limit)
SUB = SHARD // NP                      # 3125 rows per shard sub-slice
PROWS = N_CORES * SUB                  # 25000 rows per piece table
RB = 118                               # real rows per 128-row block
NBLK = -(-(NA // N_CORES) // RB)       # 106 blocks per core
OUT_ROWS = NBLK * P                    # 13568 (padded, host unshuffles)
GB = 6                                 # blocks per group (PSUM banks)

BF16 = ml_dtypes.bfloat16

# Filled by kernel() for test harness introspection.
LAST_RESULTS = None
LAST_SPMD_WALL_NS = None


def _groups():
    out = []
    b = 0
    while b < NBLK:
        out.append(list(range(b, min(b + GB, NBLK))))
        b += GB
    return out


def _host_prep(edge_rows, edge_cols, edge_vals):
    """Assign rows to (core, block) bins; sort/bin edges by (bin, piece).

    Piece of a source col c: shard s = c//12500, r = c%12500, piece =
    r//3125, row in piece table = s*3125 + r%3125.

    Chunk order: piece-major: for each piece k, for each group, for each
    block b in the group, C[b][k] chunks of 128 edge slots (C = max over
    cores).  Returns (C, per_core, row_ids) with per-core inputs:
      idx16 [16, TOT_SLOTS//16] i16 wrapped dma_gather indices (pad 0;
                                    replicated to 128 partitions on-device)
      rr8   [P, TOT_CHUNKS] u8      row-within-block per slot (pad 0)
      vv16  [P, TOT_CHUNKS] bf16    edge value per slot (pad 0)
    row_ids[d] maps core-d output slots to global rows (-1 = pad).
    """
    rows = np.asarray(edge_rows).astype(np.int64)
    cols = np.asarray(edge_cols).astype(np.int64)
    vals = np.asarray(edge_vals).astype(np.float32)

    piece = (cols % SHARD) // SUB
    prow = (cols // SHARD) * SUB + (cols % SUB)

    # per dest row: piece counts + total
    kq = np.bincount(rows * NP + piece, minlength=NA * NP).reshape(NA, NP)
    total = kq.sum(axis=1)

    # rows -> cores: snake deal by total desc
    order = np.argsort(-total, kind="stable")
    pat = np.concatenate([np.arange(N_CORES), np.arange(N_CORES)[::-1]])
    core_of = np.empty(NA, dtype=np.int64)
    core_of[order] = pat[np.arange(NA) % (2 * N_CORES)]

    # per core: rows -> blocks (stratified by dominant piece, snake deal)
    row_slot = np.empty(NA, dtype=np.int64)
    row_ids = np.full((N_CORES, OUT_ROWS), -1, dtype=np.int64)
    dom = np.argmax(kq, axis=1)
    for d in range(N_CORES):
        rd = np.where(core_of == d)[0]
        o2 = np.lexsort((-total[rd], dom[rd]))
        rd = rd[o2]
        i = np.arange(len(rd))
        j = i % (2 * NBLK)
        blk = np.where(j < NBLK, j, 2 * NBLK - 1 - j)
        occ = (i // (2 * NBLK)) * 2 + (j >= NBLK)
        slot = blk * P + occ
        assert occ.max() < P
        row_slot[rd] = slot
        row_ids[d][slot] = rd

    # edges -> (core, cell) with cell = block*NP + piece
    ecore = core_of[rows]
    eslot = row_slot[rows]
    eblk = eslot >> 7
    cell = eblk * NP + piece

    NCELL = NBLK * NP
    counts = np.zeros((N_CORES, NCELL), dtype=np.int64)
    raw = []
    for d in range(N_CORES):
        m = ecore == d
        cl = cell[m]
        o3 = np.argsort(cl, kind="stable")
        raw.append((eslot[m][o3], prow[m][o3], vals[m][o3], cl[o3]))
        counts[d] = np.bincount(cl, minlength=NCELL)

    C = (-(-counts.max(axis=0) // P)).astype(np.int64).reshape(NBLK, NP)

    # chunk base per (block, piece) in piece/group/block order
    cbase = np.zeros((NBLK, NP), dtype=np.int64)
    pos = 0
    for k in range(NP):
        for grp in _groups():
            for b in grp:
                cbase[b, k] = pos
                pos += C[b, k]
    TOT_CHUNKS = int(pos)
    TOT_SLOTS = TOT_CHUNKS * P

    per_core = []
    for d in range(N_CORES):
        es, pr, vv_e, cl = raw[d]
        cnt = counts[d]
        gstart = np.zeros(NCELL + 1, dtype=np.int64)
        np.cumsum(cnt, out=gstart[1:])
        rank = np.arange(len(es)) - gstart[cl]
        blk = cl // NP
        pk = cl % NP
        slot = (cbase[blk, pk] + (rank >> 7)) * P + (rank & 127)

        idx16 = np.zeros(TOT_SLOTS, dtype=np.int16)
        rr = np.zeros(TOT_SLOTS, dtype=np.float32)
        vv = np.zeros(TOT_SLOTS, dtype=np.float32)
        idx16[slot] = pr.astype(np.int16)
        rr[slot] = (es & 127).astype(np.float32)
        vv[slot] = vv_e

        w = idx16.reshape(TOT_SLOTS // 16, 16).T
        per_core.append({
            "idx16": w.copy(),
            "rr8": rr.reshape(TOT_CHUNKS, P).T.astype(np.uint8),
            "vv16": vv.reshape(TOT_CHUNKS, P).T.astype(BF16),
        })
    return C, per_core, row_ids


def _split_waits(nc, max_waits=1):
    """Walrus CTRL ops encode one sem wait; peel extras onto chained drains."""
    for fn in nc.m.functions:
        for bb in fn.blocks:
            new_insts = []
            for inst in bb.instructions:
                si = inst.sync_info
                if si is not None and si.on_wait and len(si.on_wait) > max_waits:
                    waits = list(si.on_wait)
                    while len(waits) > max_waits:
                        chunk, waits = waits[:max_waits], waits[max_waits:]
                        d = mybir.InstDrain(
                            name=nc.get_next_instruction_name(),
                            ins=[], outs=[], bass_is_fusable=False,
                        )
                        d.engine = inst.engine
                        d.sync_info = mybir.SyncInfo(on_wait=chunk, on_update=[])
                        nc.register_instruction(d)
                        new_insts.append(d)
                    si.on_wait = waits
                new_insts.append(inst)
            bb.instructions[:] = new_insts


def _build(C):
    C = np.asarray(C)
    groups = _groups()
    cbase = np.zeros((NBLK, NP), dtype=np.int64)
    pos = 0
    for k in range(NP):
        for grp in groups:
            for b in grp:
                cbase[b, k] = pos
                pos += int(C[b, k])
    TOT_CHUNKS = int(pos)
    TOT_SLOTS = TOT_CHUNKS * P

    f32 = mybir.dt.float32
    bf16 = mybir.dt.bfloat16
    i16 = mybir.dt.int16

    nc = bass.Bass(target_bir_lowering=False, num_swdge_queues=4)
    b_shard = nc.declare_dram_parameter("b_shard", [SHARD, F], bf16, isOutput=False)
    sub_int = []
    b_piece = []
    for k in range(NP):
        sub_int.append(nc.dram_tensor(f"sub_int{k}", [SUB, F], bf16))
        b_piece.append(nc.dram_tensor(f"b_piece{k}", [PROWS, F], bf16, addr_space="Shared"))
    w_d = nc.declare_dram_parameter("w", [F, F], bf16, isOutput=False)
    bias_d = nc.declare_dram_parameter("bias_row", [1, F], f32, isOutput=False)
    ones_d = nc.declare_dram_parameter("ones_row", [1, P], f32, isOutput=False)
    iota_d = nc.declare_dram_parameter("iota", [P, P], bf16, isOutput=False)
    idx_d = nc.declare_dram_parameter("idx16", [16, TOT_SLOTS // 16], i16, isOutput=False)
    rr_d = nc.declare_dram_parameter("rr8", [P, TOT_CHUNKS], mybir.dt.uint8, isOutput=False)
    vv_d = nc.declare_dram_parameter("vv16", [P, TOT_CHUNKS], bf16, isOutput=False)
    out_d = nc.declare_dram_parameter("out", [OUT_ROWS, F], bf16, isOutput=True)

    with tile.TileContext(nc) as tc:
        with (
            tc.tile_pool(name="const", bufs=1) as const_pool,
            tc.tile_pool(name="meta", bufs=1) as meta_pool,
            tc.tile_pool(name="acc", bufs=1) as acc_pool,
            tc.tile_pool(name="gather", bufs=5) as gather_pool,
            tc.tile_pool(name="st", bufs=10) as st_pool,
            tc.tile_pool(name="y2sb", bufs=4) as y2sb_pool,
            tc.tile_pool(name="outsb", bufs=4) as outsb_pool,
            tc.tile_pool(name="y2ps", bufs=1, space="PSUM") as y2ps_pool,
            tc.tile_pool(name="outps", bufs=2, space="PSUM") as outps_pool,
        ):
            # staged replication: 4 sub-slice AllGathers, issued in piece order
            for k in range(NP):
                nc.sync.dma_start(out=sub_int[k][:], in_=b_shard[k * SUB:(k + 1) * SUB, :])
            for k in range(NP):
                nc.gpsimd.collective_compute(
                    "AllGather",
                    mybir.AluOpType.bypass,
                    replica_groups=[list(range(N_CORES))],
                    ins=[sub_int[k][:]],
                    outs=[b_piece[k][:]],
                )
            nc.gpsimd.load_library(library_config.mlp)

            w_sb = const_pool.tile([F, F], bf16)
            bias_sb = const_pool.tile([1, F], f32)
            ones_sb = const_pool.tile([1, P], f32)
            iota_sb = const_pool.tile([P, P], bf16)
            nc.sync.dma_start(out=w_sb[:], in_=w_d[:])
            nc.sync.dma_start(out=bias_sb[:], in_=bias_d[:])
            nc.sync.dma_start(out=ones_sb[:], in_=ones_d[:])
            nc.sync.dma_start(out=iota_sb[:], in_=iota_d[:])

            sgroups = [groups[i:i + 3] for i in range(0, len(groups), 3)]
            ni_vals = sorted({
                int(sum(C[b, k] for grp in sg for b in grp)) * P
                for sg in sgroups for k in range(NP)
                if sum(C[b, k] for grp in sg for b in grp) > 0
            })
            ni_regs = {}
            for v in ni_vals:
                ni_r = nc.gpsimd.alloc_register(f"ni_{v}")
                nc.gpsimd.reg_mov(ni_r, v)
                ni_regs[v] = ni_r

            idx_sb = meta_pool.tile([P, TOT_SLOTS // 16], i16)
            rr8_sb = meta_pool.tile([P, TOT_CHUNKS], mybir.dt.uint8)
            vv16_sb = meta_pool.tile([P, TOT_CHUNKS], bf16)
            rr_sb = meta_pool.tile([P, TOT_CHUNKS], f32)
            vv_sb = meta_pool.tile([P, TOT_CHUNKS], f32)
            for rep in range(8):
                nc.sync.dma_start(
                    out=idx_sb[rep * 16:(rep + 1) * 16, :], in_=idx_d[:])
            nc.sync.dma_start(out=rr8_sb[:], in_=rr_d[:])
            nc.sync.dma_start(out=vv16_sb[:], in_=vv_d[:])
            nc.vector.tensor_copy(out=rr_sb[:], in_=rr8_sb[:])
            nc.vector.tensor_copy(out=vv_sb[:], in_=vv16_sb[:])

            acc_tiles = []
            for b in range(NBLK):
                acc_t = acc_pool.tile([F, P], bf16, tag=f"acc_{b}", name=f"acc{b}")
                acc_tiles.append(acc_t)
            # first/last non-empty piece per block (the flush schedule)
            first_k = {}
            last_k = {}
            for b in range(NBLK):
                nz = [k for k in range(NP) if int(C[b, k]) > 0]
                first_k[b] = nz[0] if nz else -1
                last_k[b] = nz[-1] if nz else -1

            y2sb_last = {}

            def finalize(b):
                o_ps = outps_pool.tile([P, F], f32, tag="ops", name="o_ps")
                nc.tensor.matmul(
                    out=o_ps[:], lhsT=ones_sb[:], rhs=bias_sb[:],
                    start=True, stop=False,
                )
                nc.tensor.matmul(
                    out=o_ps[:], lhsT=y2sb_last[b][:], rhs=w_sb[:],
                    start=False, stop=True,
                )
                o_sb = outsb_pool.tile([P, F], bf16, tag="osb", name="o_sb")
                nc.scalar.activation(
                    out=o_sb[:], in_=o_ps[:],
                    func=mybir.ActivationFunctionType.Copy,
                )
                nc.sync.dma_start(out=out_d[b * P:(b + 1) * P, :], in_=o_sb[:])

            for k in range(NP):
                for sg in sgroups:
                  nchunks_sg = int(sum(C[b, k] for grp in sg for b in grp))
                  sgpos0 = int(cbase[sg[0][0], k])
                  if nchunks_sg > 0:
                    g_t = gather_pool.tile([P, nchunks_sg * F], bf16, tag="g",
                                           name="g_t")
                    nc.gpsimd.dma_gather(
                        g_t[:].rearrange("p (c f) -> p c f", f=F),
                        b_piece[k][:],
                        idx_sb[:, sgpos0 * 8:(sgpos0 + nchunks_sg) * 8],
                        nchunks_sg * P,
                        ni_regs[nchunks_sg * P],
                        F,
                        single_packet=False,
                        queue_num=0,
                    )
                  for grp in sg:
                    gpos0 = sgpos0
                    for bi, b in enumerate(grp):
                        nch = int(C[b, k])
                        if nch == 0:
                            if k == NP - 1 and last_k[b] != NP - 1:
                                # block's chunks all landed in earlier
                                # pieces: flush acc directly
                                y2_sb = y2sb_pool.tile([F, P], bf16, tag="y2sb",
                                                       name="y2_sb")
                                if last_k[b] >= 0:
                                    nc.scalar.activation(
                                        out=y2_sb[:], in_=acc_tiles[b][:],
                                        func=mybir.ActivationFunctionType.Copy,
                                    )
                                else:
                                    nc.vector.memset(y2_sb[:], 0.0)
                                y2sb_last[b] = y2_sb
                                finalize(b)
                            continue
                        y2 = y2ps_pool.tile([F, P], f32, tag=f"y2s_{bi}",
                                            name=f"y2_{bi}")
                        for j in range(nch):
                            gpos = int(cbase[b, k]) + j
                            goff = (gpos - gpos0) * F
                            s_t = st_pool.tile([P, P], bf16, tag="s_t", name="s_t")
                            nc.vector.tensor_scalar(
                                out=s_t[:],
                                in0=iota_sb[:],
                                scalar1=rr_sb[:, gpos:gpos + 1],
                                scalar2=vv_sb[:, gpos:gpos + 1],
                                op0=mybir.AluOpType.is_equal,
                                op1=mybir.AluOpType.mult,
                            )
                            nc.tensor.matmul(
                                out=y2[:],
                                lhsT=g_t[:, goff:goff + F],
                                rhs=s_t[:],
                                start=(j == 0),
                                stop=(j == nch - 1),
                            )
                        if k == last_k[b]:
                            y2_sb = y2sb_pool.tile([F, P], bf16, tag="y2sb",
                                                   name="y2_sb")
                            if k == first_k[b]:
                                nc.scalar.activation(
                                    out=y2_sb[:], in_=y2[:],
                                    func=mybir.ActivationFunctionType.Copy,
                                )
                            else:
                                nc.vector.tensor_tensor(
                                    out=y2_sb[:], in0=y2[:],
                                    in1=acc_tiles[b][:], op=mybir.AluOpType.add,
                                )
                            y2sb_last[b] = y2_sb
                            finalize(b)
                        elif k == first_k[b]:
                            nc.scalar.activation(
                                out=acc_tiles[b][:], in_=y2[:],
                                func=mybir.ActivationFunctionType.Copy,
                            )
                        else:
                            nc.vector.tensor_tensor(
                                out=acc_tiles[b][:], in0=y2[:],
                                in1=acc_tiles[b][:], op=mybir.AluOpType.add,
                            )
    nc.finalize()
    lower_extended_insts(nc)
    _split_waits(nc)
    return nc


def kernel(b_input, edge_rows, edge_cols, edge_vals, a_weight, a_bias):
    global LAST_RESULTS
    b_input = np.ascontiguousarray(np.asarray(b_input, dtype=np.float32))
    a_weight = np.ascontiguousarray(np.asarray(a_weight, dtype=np.float32))
    a_bias = np.asarray(a_bias, dtype=np.float32)

    C, per_core, row_ids = _host_prep(edge_rows, edge_cols, edge_vals)
    nc = _build(C)

    b16 = b_input.astype(BF16)
    w16 = a_weight.astype(BF16)
    bias_row = a_bias[None, :].astype(np.float32)
    ones_row = np.ones((1, P), dtype=np.float32)
    iota = np.tile(np.arange(P, dtype=np.float32)[None, :], (P, 1)).astype(BF16)

    in_maps = []
    for d in range(N_CORES):
        in_maps.append({
            "b_shard": b16[d * SHARD:(d + 1) * SHARD],
            "w": w16,
            "bias_row": bias_row,
            "ones_row": ones_row,
            "iota": iota,
            "idx16": per_core[d]["idx16"],
            "rr8": per_core[d]["rr8"],
            "vv16": per_core[d]["vv16"],
        })

    import time as _time
    global LAST_SPMD_WALL_NS
    _t0 = _time.time()
    res = run_bass_kernel_spmd(nc, in_maps, core_ids=list(range(N_CORES)))
    LAST_SPMD_WALL_NS = int((_time.time() - _t0) * 1e9)
    LAST_RESULTS = res

    out = np.empty((NA, F), dtype=np.float32)
    for d in range(N_CORES):
        ids = row_ids[d]
        valid = ids >= 0
        out[ids[valid]] = res.results[d]["out"][valid].astype(np.float32)
    return out


# revision 22
# speedup vs baseline: 1.0626x; 1.0626x over previous
"""BiGraphConv (gather + SpMM segment-sum + linear) on 8 Trainium2 NeuronCores.

Strategy (1D output-row partition, staged replication):
  - Output rows are assigned to (core, block) bins on the host with a
    stratified deal (118 rows per 128-row block), equalizing per-(block,
    piece) edge counts across cores so the SPMD-uniform chunk counts
    carry ~8% padding instead of ~25%.  Each core owns its bins' edges;
    no inter-core reduction is needed.  The host unshuffles the output.
  - b_input is cast to bf16 on the host and uploaded sharded (1/8 per
    core).  Replication is staged: the table is split into 4 "pieces"
    (piece k = concat of every shard's k-th 3125-row sub-slice, 25000
    rows each, small enough for dma_gather's int16 indices), each
    materialized by its own AllGather.  Compute is piece-major, so
    piece-0 compute overlaps the remaining collectives.
  - Edge-row gathers use the GPSIMD dma_gather custom instruction (mlp
    ucode library): one op fetches ~3k 256B rows, amortizing the ~1us
    SWDGE descriptor-generation fixed cost.  Only gathers run on the
    Pool/SWDGE path; every regular DMA uses HWDGE.
  - Per piece, each block accumulates its chunks in its own PSUM bank
    (matmul start/stop state is per-bank: concurrent chains must not
    share one), then flushes into a per-block SBUF bf16 accumulator
    (ScalarE copy on piece 0, VectorE add after); the last flush writes
    bf16 and feeds the weight matmul directly: out[r,f] = Y2.T @ W.
  - Segment-sum via TensorE in bf16: per chunk of 128 edges,
    Y2[c,r] += sum_e G[e,c] * S_T[e,r] with S_T[e,r] = val_e*(row_e == r)
    built on VectorE with one fused tensor_scalar (is_equal then mult).
  - The bias rides the output PSUM accumulation as a rank-1 matmul
    (ones x bias); the output needs only a ScalarE copy (as bf16) before
    the store, and the host upcasts to f32.

kernel(**inputs) takes the FULL inputs and returns the FULL [100000,128]
output.  Self-contained: shapes/sharding are hardcoded.
"""

import numpy as np
import ml_dtypes

import concourse.bass as bass
import concourse.mybir as mybir
import concourse.tile as tile
from concourse import library_config
from concourse.bass_utils import run_bass_kernel_spmd
from concourse.library_overlay import lower_extended_insts

NA = 100000
NB = 100000
NE = 1600000
F = 128          # feature dim (both sides)
P = 128          # partitions / block rows / chunk size
N_CORES = 8
SHARD = NB // N_CORES                  # 12500 table rows per core
NP = 4                                 # table pieces (int16 idx limit)
SUB = SHARD // NP                      # 3125 rows per shard sub-slice
PROWS = N_CORES * SUB                  # 25000 rows per piece table
RB = 118                               # real rows per 128-row block
NBLK = -(-(NA // N_CORES) // RB)       # 106 blocks per core
OUT_ROWS = NBLK * P                    # 13568 (padded, host unshuffles)
GB = 6                                 # blocks per group (PSUM banks)

BF16 = ml_dtypes.bfloat16

# Filled by kernel() for test harness introspection.
LAST_RESULTS = None
LAST_SPMD_WALL_NS = None


def _groups():
    out = []
    b = 0
    while b < NBLK:
        out.append(list(range(b, min(b + GB, NBLK))))
        b += GB
    return out


def _host_prep(edge_rows, edge_cols, edge_vals):
    """Assign rows to (core, block) bins; sort/bin edges by (bin, piece).

    Piece of a source col c: shard s = c//12500, r = c%12500, piece =
    r//3125, row in piece table = s*3125 + r%3125.

    Chunk order: piece-major: for each piece k, for each group, for each
    block b in the group, C[b][k] chunks of 128 edge slots (C = max over
    cores).  Returns (C, per_core, row_ids) with per-core inputs:
      idx16 [16, TOT_SLOTS//16] i16 wrapped dma_gather indices (pad 0;
                                    replicated to 128 partitions on-device)
      rr8   [P, TOT_CHUNKS] u8      row-within-block per slot (pad 0)
      vv16  [P, TOT_CHUNKS] bf16    edge value per slot (pad 0)
    row_ids[d] maps core-d output slots to global rows (-1 = pad).
    """
    rows = np.asarray(edge_rows).astype(np.int64)
    cols = np.asarray(edge_cols).astype(np.int64)
    vals = np.asarray(edge_vals).astype(np.float32)

    piece = (cols % SHARD) // SUB
    prow = (cols // SHARD) * SUB + (cols % SUB)

    # per dest row: piece counts + total
    kq = np.bincount(rows * NP + piece, minlength=NA * NP).reshape(NA, NP)
    total = kq.sum(axis=1)

    # rows -> cores: snake deal by total desc
    order = np.argsort(-total, kind="stable")
    pat = np.concatenate([np.arange(N_CORES), np.arange(N_CORES)[::-1]])
    core_of = np.empty(NA, dtype=np.int64)
    core_of[order] = pat[np.arange(NA) % (2 * N_CORES)]

    # per core: rows -> blocks (stratified by dominant piece, snake deal)
    row_slot = np.empty(NA, dtype=np.int64)
    row_ids = np.full((N_CORES, OUT_ROWS), -1, dtype=np.int64)
    dom = np.argmax(kq, axis=1)
    for d in range(N_CORES):
        rd = np.where(core_of == d)[0]
        o2 = np.lexsort((-total[rd], dom[rd]))
        rd = rd[o2]
        i = np.arange(len(rd))
        j = i % (2 * NBLK)
        blk = np.where(j < NBLK, j, 2 * NBLK - 1 - j)
        occ = (i // (2 * NBLK)) * 2 + (j >= NBLK)
        slot = blk * P + occ
        assert occ.max() < P
        row_slot[rd] = slot
        row_ids[d][slot] = rd

    # edges -> (core, cell) with cell = block*NP + piece
    ecore = core_of[rows]
    eslot = row_slot[rows]
    eblk = eslot >> 7
    cell = eblk * NP + piece

    NCELL = NBLK * NP
    counts = np.zeros((N_CORES, NCELL), dtype=np.int64)
    raw = []
    for d in range(N_CORES):
        m = ecore == d
        cl = cell[m]
        o3 = np.argsort(cl, kind="stable")
        raw.append((eslot[m][o3], prow[m][o3], vals[m][o3], cl[o3]))
        counts[d] = np.bincount(cl, minlength=NCELL)

    C = (-(-counts.max(axis=0) // P)).astype(np.int64).reshape(NBLK, NP)

    # chunk base per (block, piece) in piece/group/block order
    cbase = np.zeros((NBLK, NP), dtype=np.int64)
    pos = 0
    for k in range(NP):
        for grp in _groups():
            for b in grp:
                cbase[b, k] = pos
                pos += C[b, k]
    TOT_CHUNKS = int(pos)
    TOT_SLOTS = TOT_CHUNKS * P

    per_core = []
    for d in range(N_CORES):
        es, pr, vv_e, cl = raw[d]
        cnt = counts[d]
        gstart = np.zeros(NCELL + 1, dtype=np.int64)
        np.cumsum(cnt, out=gstart[1:])
        rank = np.arange(len(es)) - gstart[cl]
        blk = cl // NP
        pk = cl % NP
        slot = (cbase[blk, pk] + (rank >> 7)) * P + (rank & 127)

        idx16 = np.zeros(TOT_SLOTS, dtype=np.int16)
        rr = np.zeros(TOT_SLOTS, dtype=np.float32)
        vv = np.zeros(TOT_SLOTS, dtype=np.float32)
        idx16[slot] = pr.astype(np.int16)
        rr[slot] = (es & 127).astype(np.float32)
        vv[slot] = vv_e

        w = idx16.reshape(TOT_SLOTS // 16, 16).T
        per_core.append({
            "idx16": w.copy(),
            "rr8": rr.reshape(TOT_CHUNKS, P).T.astype(np.uint8),
            "vv16": vv.reshape(TOT_CHUNKS, P).T.astype(BF16),
        })
    return C, per_core, row_ids


def _split_waits(nc, max_waits=1):
    """Walrus CTRL ops encode one sem wait; peel extras onto chained drains."""
    for fn in nc.m.functions:
        for bb in fn.blocks:
            new_insts = []
            for inst in bb.instructions:
                si = inst.sync_info
                if si is not None and si.on_wait and len(si.on_wait) > max_waits:
                    waits = list(si.on_wait)
                    while len(waits) > max_waits:
                        chunk, waits = waits[:max_waits], waits[max_waits:]
                        d = mybir.InstDrain(
                            name=nc.get_next_instruction_name(),
                            ins=[], outs=[], bass_is_fusable=False,
                        )
                        d.engine = inst.engine
                        d.sync_info = mybir.SyncInfo(on_wait=chunk, on_update=[])
                        nc.register_instruction(d)
                        new_insts.append(d)
                    si.on_wait = waits
                new_insts.append(inst)
            bb.instructions[:] = new_insts


def _build(C):
    C = np.asarray(C)
    groups = _groups()
    cbase = np.zeros((NBLK, NP), dtype=np.int64)
    pos = 0
    for k in range(NP):
        for grp in groups:
            for b in grp:
                cbase[b, k] = pos
                pos += int(C[b, k])
    TOT_CHUNKS = int(pos)
    TOT_SLOTS = TOT_CHUNKS * P

    f32 = mybir.dt.float32
    bf16 = mybir.dt.bfloat16
    i16 = mybir.dt.int16

    nc = bass.Bass(target_bir_lowering=False, num_swdge_queues=4)
    b_shard = nc.declare_dram_parameter("b_shard", [SHARD, F], bf16, isOutput=False)
    sub_int = []
    b_piece = []
    for k in range(NP):
        sub_int.append(nc.dram_tensor(f"sub_int{k}", [SUB, F], bf16))
        b_piece.append(nc.dram_tensor(f"b_piece{k}", [PROWS, F], bf16, addr_space="Shared"))
    w_d = nc.declare_dram_parameter("w", [F, F], bf16, isOutput=False)
    bias_d = nc.declare_dram_parameter("bias_row", [1, F], f32, isOutput=False)
    ones_d = nc.declare_dram_parameter("ones_row", [1, P], f32, isOutput=False)
    iota_d = nc.declare_dram_parameter("iota", [P, P], bf16, isOutput=False)
    idx_d = nc.declare_dram_parameter("idx16", [16, TOT_SLOTS // 16], i16, isOutput=False)
    rr_d = nc.declare_dram_parameter("rr8", [P, TOT_CHUNKS], mybir.dt.uint8, isOutput=False)
    vv_d = nc.declare_dram_parameter("vv16", [P, TOT_CHUNKS], bf16, isOutput=False)
    out_d = nc.declare_dram_parameter("out", [OUT_ROWS, F], bf16, isOutput=True)

    with tile.TileContext(nc) as tc:
        with (
            tc.tile_pool(name="const", bufs=1) as const_pool,
            tc.tile_pool(name="meta", bufs=1) as meta_pool,
            tc.tile_pool(name="acc", bufs=1) as acc_pool,
            tc.tile_pool(name="gather", bufs=5) as gather_pool,
            tc.tile_pool(name="st", bufs=10) as st_pool,
            tc.tile_pool(name="y2sb", bufs=4) as y2sb_pool,
            tc.tile_pool(name="outsb", bufs=4) as outsb_pool,
            tc.tile_pool(name="y2ps", bufs=1, space="PSUM") as y2ps_pool,
            tc.tile_pool(name="outps", bufs=2, space="PSUM") as outps_pool,
        ):
            # staged replication: 4 sub-slice AllGathers, issued in piece order
            for k in range(NP):
                nc.sync.dma_start(out=sub_int[k][:], in_=b_shard[k * SUB:(k + 1) * SUB, :])
            for k in range(NP):
                nc.gpsimd.collective_compute(
                    "AllGather",
                    mybir.AluOpType.bypass,
                    replica_groups=[list(range(N_CORES))],
                    ins=[sub_int[k][:]],
                    outs=[b_piece[k][:]],
                )
            nc.gpsimd.load_library(library_config.mlp)

            w_sb = const_pool.tile([F, F], bf16)
            bias_sb = const_pool.tile([1, F], f32)
            ones_sb = const_pool.tile([1, P], f32)
            iota_sb = const_pool.tile([P, P], bf16)
            nc.sync.dma_start(out=w_sb[:], in_=w_d[:])
            nc.sync.dma_start(out=bias_sb[:], in_=bias_d[:])
            nc.sync.dma_start(out=ones_sb[:], in_=ones_d[:])
            nc.sync.dma_start(out=iota_sb[:], in_=iota_d[:])

            sgroups = [groups[i:i + 3] for i in range(0, len(groups), 3)]
            ni_vals = sorted({
                int(sum(C[b, k] for grp in sg for b in grp)) * P
                for sg in sgroups for k in range(NP)
                if sum(C[b, k] for grp in sg for b in grp) > 0
            })
            ni_regs = {}
            for v in ni_vals:
                ni_r = nc.gpsimd.alloc_register(f"ni_{v}")
                nc.gpsimd.reg_mov(ni_r, v)
                ni_regs[v] = ni_r

            idx_sb = meta_pool.tile([P, TOT_SLOTS // 16], i16)
            rr8_sb = meta_pool.tile([P, TOT_CHUNKS], mybir.dt.uint8)
            vv16_sb = meta_pool.tile([P, TOT_CHUNKS], bf16)
            rr_sb = meta_pool.tile([P, TOT_CHUNKS], f32)
            vv_sb = meta_pool.tile([P, TOT_CHUNKS], f32)
            for rep in range(8):
                nc.sync.dma_start(
                    out=idx_sb[rep * 16:(rep + 1) * 16, :], in_=idx_d[:])
            nc.sync.dma_start(out=rr8_sb[:], in_=rr_d[:])
            nc.sync.dma_start(out=vv16_sb[:], in_=vv_d[:])
            nc.vector.tensor_copy(out=rr_sb[:], in_=rr8_sb[:])
            nc.vector.tensor_copy(out=vv_sb[:], in_=vv16_sb[:])

            acc_tiles = []
            for b in range(NBLK):
                acc_t = acc_pool.tile([F, P], bf16, tag=f"acc_{b}", name=f"acc{b}")
                acc_tiles.append(acc_t)
            # first/last non-empty piece per block (the flush schedule)
            first_k = {}
            last_k = {}
            for b in range(NBLK):
                nz = [k for k in range(NP) if int(C[b, k]) > 0]
                first_k[b] = nz[0] if nz else -1
                last_k[b] = nz[-1] if nz else -1

            y2sb_last = {}

            def finalize(b):
                o_ps = outps_pool.tile([P, F], f32, tag="ops", name="o_ps")
                nc.tensor.matmul(
                    out=o_ps[:], lhsT=ones_sb[:], rhs=bias_sb[:],
                    start=True, stop=False,
                )
                nc.tensor.matmul(
                    out=o_ps[:], lhsT=y2sb_last[b][:], rhs=w_sb[:],
                    start=False, stop=True,
                )
                o_sb = outsb_pool.tile([P, F], bf16, tag="osb", name="o_sb")
                nc.scalar.activation(
                    out=o_sb[:], in_=o_ps[:],
                    func=mybir.ActivationFunctionType.Copy,
                )
                nc.sync.dma_start(out=out_d[b * P:(b + 1) * P, :], in_=o_sb[:])

            for k in range(NP):
                for sg in sgroups:
                  nchunks_sg = int(sum(C[b, k] for grp in sg for b in grp))
                  sgpos0 = int(cbase[sg[0][0], k])
                  if nchunks_sg > 0:
                    g_t = gather_pool.tile([P, nchunks_sg * F], bf16, tag="g",
                                           name="g_t")
                    nc.gpsimd.dma_gather(
                        g_t[:].rearrange("p (c f) -> p c f", f=F),
                        b_piece[k][:],
                        idx_sb[:, sgpos0 * 8:(sgpos0 + nchunks_sg) * 8],
                        nchunks_sg * P,
                        ni_regs[nchunks_sg * P],
                        F,
                        single_packet=False,
                        queue_num=0,
                    )
                  for grp in sg:
                    gpos0 = sgpos0
                    for bi, b in enumerate(grp):
                        nch = int(C[b, k])
                        if nch == 0:
                            if k == NP - 1 and last_k[b] != NP - 1:
                                # block's chunks all landed in earlier
                                # pieces: flush acc directly
                                y2_sb = y2sb_pool.tile([F, P], bf16, tag="y2sb",
                                                       name="y2_sb")
                                if last_k[b] >= 0:
                                    nc.scalar.activation(
                                        out=y2_sb[:], in_=acc_tiles[b][:],
                                        func=mybir.ActivationFunctionType.Copy,
                                    )
                                else:
                                    nc.vector.memset(y2_sb[:], 0.0)
                                y2sb_last[b] = y2_sb
                                finalize(b)
                            continue
                        y2 = y2ps_pool.tile([F, P], f32, tag=f"y2s_{bi}",
                                            name=f"y2_{bi}")
                        for j in range(nch):
                            gpos = int(cbase[b, k]) + j
                            goff = (gpos - gpos0) * F
                            s_t = st_pool.tile([P, P], bf16, tag="s_t", name="s_t")
                            nc.vector.tensor_scalar(
                                out=s_t[:],
                                in0=iota_sb[:],
                                scalar1=rr_sb[:, gpos:gpos + 1],
                                scalar2=vv_sb[:, gpos:gpos + 1],
                                op0=mybir.AluOpType.is_equal,
                                op1=mybir.AluOpType.mult,
                            )
                            nc.tensor.matmul(
                                out=y2[:],
                                lhsT=g_t[:, goff:goff + F],
                                rhs=s_t[:],
                                start=(j == 0),
                                stop=(j == nch - 1),
                            )
                        if k == last_k[b]:
                            y2_sb = y2sb_pool.tile([F, P], bf16, tag="y2sb",
                                                   name="y2_sb")
                            if k == first_k[b]:
                                nc.scalar.activation(
                                    out=y2_sb[:], in_=y2[:],
                                    func=mybir.ActivationFunctionType.Copy,
                                )
                            else:
                                nc.vector.tensor_tensor(
                                    out=y2_sb[:], in0=y2[:],
                                    in1=acc_tiles[b][:], op=mybir.AluOpType.add,
                                )
                            y2sb_last[b] = y2_sb
                            finalize(b)
                        elif k == first_k[b]:
                            nc.scalar.activation(
                                out=acc_tiles[b][:], in_=y2[:],
                                func=mybir.ActivationFunctionType.Copy,
                            )
                        else:
                            nc.vector.tensor_tensor(
                                out=acc_tiles[b][:], in0=y2[:],
                                in1=acc_tiles[b][:], op=mybir.AluOpType.add,
                            )
    nc.finalize()
    lower_extended_insts(nc)
    _split_waits(nc)
    return nc


def kernel(b_input, edge_rows, edge_cols, edge_vals, a_weight, a_bias):
    global LAST_RESULTS
    b_input = np.ascontiguousarray(np.asarray(b_input, dtype=np.float32))
    a_weight = np.ascontiguousarray(np.asarray(a_weight, dtype=np.float32))
    a_bias = np.asarray(a_bias, dtype=np.float32)

    C, per_core, row_ids = _host_prep(edge_rows, edge_cols, edge_vals)
    nc = _build(C)

    b16 = b_input.astype(BF16)
    w16 = a_weight.astype(BF16)
    bias_row = a_bias[None, :].astype(np.float32)
    ones_row = np.ones((1, P), dtype=np.float32)
    iota = np.tile(np.arange(P, dtype=np.float32)[None, :], (P, 1)).astype(BF16)

    in_maps = []
    for d in range(N_CORES):
        in_maps.append({
            "b_shard": b16[d * SHARD:(d + 1) * SHARD],
            "w": w16,
            "bias_row": bias_row,
            "ones_row": ones_row,
            "iota": iota,
            "idx16": per_core[d]["idx16"],
            "rr8": per_core[d]["rr8"],
            "vv16": per_core[d]["vv16"],
        })

    import time as _time
    global LAST_SPMD_WALL_NS
    _t0 = _time.time()
    res = run_bass_kernel_spmd(nc, in_maps, core_ids=list(range(N_CORES)))
    LAST_SPMD_WALL_NS = int((_time.time() - _t0) * 1e9)
    LAST_RESULTS = res

    out = np.empty((NA, F), dtype=np.float32)
    for d in range(N_CORES):
        ids = row_ids[d]
        valid = ids >= 0
        out[ids[valid]] = res.results[d]["out"][valid].astype(np.float32)
    return out
